# revision 11
# baseline (speedup 1.0000x reference)
"""Trainium2 Bass kernel for nn_CantorModalityFusion.

Sharding: 8 cores = (batch b in 0..3) x (position parity in 0..1).
Each core handles batch b, positions s = par, par+2, ... (1024 positions).
Per-(b, s) independent -> no collectives.

v2: the modality projection is folded into QKV on the host:
  q_m = x_m @ (Wq @ W_m).T + [Wq @ (b_m + emb_m) + bq]
so there is no stage A at all (except the constant folding); q/k/v are
computed directly from x.T with host-premultiplied weights in bf16
(fp32 PSUM accumulation).  This removes ~37% of PE cycles vs the
staged formulation and removes the stageA->QKV serialization.

Pipeline per 512-position block:
  P1: q.T/k.T per output chunk c directly from x; s_w += sel.T @ (q*k)
  SM: softmax over the 3 routed windows                      [DVE+ACT]
  P2: v.T per chunk; A16_r = sum attn; Abc = bcast(A16);
      fused.T[c] = sum_r Abc_r * v.T[r]                      [PE+DVE]
  D:  y.T = Wo.T.T @ fused.T (+ bo)   (deferred one block)   [PE+ACT]
"""

import sys

import numpy as np

sys.path.insert(0, "/opt/trn_rl_repo")

import concourse.bacc as bacc
import concourse.mybir as mybir
from concourse import tile
from concourse.bass_utils import run_bass_kernel_spmd

F32 = mybir.dt.float32
F32R = mybir.dt.float32r
BF16 = mybir.dt.bfloat16
BF16_NP = mybir.dt.np(mybir.dt.bfloat16)
AF = mybir.ActivationFunctionType
ALU = mybir.AluOpType

B, S, D, H, HD = 4, 2048, 1024, 16, 64
M, WIN = 4, 3
MOD = [("text", 768, 2048), ("image", 1024, 1024), ("audio", 512, 1500), ("video", 2048, 512)]
ROUTES = [[0, 1, 2], [0, 1, 2], [2, 3, 0], [3, 2, 0]]
PAIRS = [(m, w, ROUTES[m][w]) for m in range(M) for w in range(WIN)]
SRC = {r: [(m, w) for (m, w, rr) in PAIRS if rr == r] for r in range(M)}
PAIR_IDX = {(m, w): m * WIN + w for m in range(M) for w in range(WIN)}

NPOS = S // 2
BLK = 512
NBLK = NPOS // BLK
NCH = D // 128                        # 8 output feature chunks
NK = [MOD[m][1] // 128 for m in range(M)]     # input chunks: 6, 8, 4, 16
NLOC = [sl // 2 for (_, _, sl) in MOD]        # 1024, 512, 750, 256
NLOCP = [1024, 512, 768, 256]                 # audio padded to 768 cols

_BUILD_CACHE = {}


def block_cfg(blk):
    """Per-block list of (m, na_mm, nab): matmul cols and bias cols."""
    out = []
    for m in range(M):
        nab = max(0, min(BLK, NLOC[m] - blk * BLK))       # true active
        na_mm = max(0, min(BLK, NLOCP[m] - blk * BLK))    # padded (>=256)
        if nab > 0:
            out.append((m, na_mm, nab))
    # video last: its weight tiles are the biggest
    out.sort(key=lambda t: (t[0] == 3, t[0]))
    return out


def build(scale, repeat=1):
    key = (float(scale), repeat)
    if key in _BUILD_CACHE:
        return _BUILD_CACHE[key]
    nc = bacc.Bacc("TRN2", target_bir_lowering=False, debug=False)

    MDT = [F32R, BF16, F32R, BF16]
    xT = [nc.dram_tensor(f"xT{m}", [MOD[m][1], NLOCP[m]], MDT[m],
                         kind="ExternalInput") for m in range(M)]
    DQK = [nc.dram_tensor(f"DQK{m}", [NCH, 128, 2, NK[m], 128], MDT[m],
                          kind="ExternalInput") for m in range(M)]
    DV = [nc.dram_tensor(f"DV{m}", [NCH, 128, NK[m], 128], MDT[m],
                         kind="ExternalInput") for m in range(M)]
    WoT = nc.dram_tensor("WoT", [NCH, 128, NCH, 128], BF16, kind="ExternalInput")
    cq_d = nc.dram_tensor("cq", [128, M, NCH], F32, kind="ExternalInput")
    ck_d = nc.dram_tensor("ck", [128, M, NCH], F32, kind="ExternalInput")
    cv_d = nc.dram_tensor("cv", [128, M, NCH], F32, kind="ExternalInput")
    bo_d = nc.dram_tensor("bo", [128, NCH], F32, kind="ExternalInput")
    selw_d = nc.dram_tensor("selw", [128, 127], F32R, kind="ExternalInput")
    selA_d = nc.dram_tensor("selA", [64, M * WIN, 16], BF16, kind="ExternalInput")
    selB_d = nc.dram_tensor("selB", [16, NCH, 128], BF16, kind="ExternalInput")
    yT = nc.dram_tensor("yT", [D, NPOS], F32, kind="ExternalOutput")

    with tile.TileContext(nc) as tc:
        with (
            tc.tile_pool(name="const", bufs=1) as cpool,
            tc.tile_pool(name="wqk", bufs=1) as wqkpool,
            tc.tile_pool(name="wv", bufs=1) as wvpool,
            tc.tile_pool(name="wo", bufs=8) as wopool,
            tc.tile_pool(name="xt", bufs=1) as xtpool,
            tc.tile_pool(name="qk", bufs=1) as qkpool,
            tc.tile_pool(name="pr", bufs=1) as prpool,
            tc.tile_pool(name="sm", bufs=1) as smpool,
            tc.tile_pool(name="fz", bufs=1) as fzpool,
            tc.tile_pool(name="yo", bufs=2) as yopool,
            tc.tile_pool(name="ps", bufs=1, space="PSUM") as pspool,
        ):
            def psum(tag, shape=(128, BLK)):
                return pspool.tile(list(shape), F32, tag=f"a{tag}", name=f"ps_a{tag}")

            # ---- constants ----
            selw = cpool.tile([128, 127], F32R, tag="selw")
            nc.sync.dma_start(selw[:], selw_d[:])
            selA = cpool.tile([64, M * WIN, 16], BF16, tag="selA")
            nc.sync.dma_start(selA[:], selA_d[:])
            selB = cpool.tile([16, NCH, 128], BF16, tag="selB")
            nc.sync.dma_start(selB[:], selB_d[:])
            cqkv = {}
            for nm, dd in (("cq", cq_d), ("ck", ck_d), ("cv", cv_d)):
                t = cpool.tile([128, M, NCH], F32, tag=nm)
                nc.sync.dma_start(t[:], dd[:])
                cqkv[nm] = t
            bo_t = cpool.tile([128, NCH], F32, tag="bo")
            nc.sync.dma_start(bo_t[:], bo_d[:])

            dma_engs = [nc.sync, nc.scalar, nc.gpsimd]

            import contextlib
            rep_cm = (tc.For_i(0, repeat, 1,
                               hint_engines=(mybir.EngineType.PE,
                                             mybir.EngineType.Activation,
                                             mybir.EngineType.DVE,
                                             mybir.EngineType.SP,
                                             mybir.EngineType.Pool))
                      if repeat > 1 else contextlib.nullcontext())
            pending_D = []
            rot = [0]

            def nxt_acc(shape=(128, BLK)):
                rot[0] += 1
                return psum(rot[0] % 5, shape)

            with rep_cm:
                for blk in range(NBLK):
                    cfg = block_cfg(blk)
                    act_m = [m for (m, _, _) in cfg]
                    act_set = set(act_m)
                    act_pairs = [(m, w, r) for (m, w, r) in PAIRS
                                 if m in act_set and r in act_set]
                    act_r = sorted({r for (_, _, r) in act_pairs})
                    p0 = blk * BLK

                    # ---- x tiles for this block ----
                    xt = {}
                    ei = 0
                    for (m, na_mm, _) in cfg:
                        for dk in range(NK[m]):
                            t = xtpool.tile([128, na_mm], MDT[m], tag=f"x{m}_{dk}",
                                            name=f"x{m}_{dk}")
                            dma_engs[ei % 3].dma_start(
                                t[:], xT[m][dk * 128:(dk + 1) * 128, p0:p0 + na_mm])
                            ei += 1
                            xt[(m, dk)] = t

                    # ---------- pass 1: q, k, scores ----------
                    n_sc = {w: sum(1 for (mm, w2, rr) in act_pairs if w2 == w) * NCH
                            for w in range(WIN)}
                    c_sc = {w: 0 for w in range(WIN)}
                    sc_ps = [psum(5 + w, (64, BLK)) if n_sc[w] > 0 else None
                             for w in range(WIN)]

                    def emit_qk(c):
                        qk_t = {}
                        for (m, na_mm, nab) in cfg:
                            wqk = wqkpool.tile([128, 2, NK[m], 128], MDT[m],
                                               bufs=(2 if m in (0, 1, 2) else 1),
                                               tag=f"wqk{m}", name=f"wqk{m}")
                            dma_engs[(c + m) % 3].dma_start(wqk[:], DQK[m][c])
                            for ti, (tname, cn) in enumerate(
                                    (("q", "cq"), ("k", "ck"))):
                                acc = nxt_acc()
                                for dk in range(NK[m]):
                                    nc.tensor.matmul(
                                        acc[:, :na_mm], wqk[:, ti, dk, :],
                                        xt[(m, dk)][:, :na_mm],
                                        start=(dk == 0), stop=(dk == NK[m] - 1),
                                        skip_group_check=True)
                                t = qkpool.tile([128, BLK], F32,
                                                tag=f"{tname}{m}_{c % 2}",
                                                name=f"{tname}{m}")
                                nc.scalar.activation(
                                    t[:, :nab], acc[:, :nab], AF.Identity,
                                    bias=cqkv[cn][:, m, c:c + 1])
                                if nab < BLK:
                                    nc.vector.memset(t[:, nab:BLK], 0.0)
                                qk_t[(tname, m)] = t
                        return qk_t

                    def emit_scores(c, qk_t):
                        for pi, (m, w, r) in enumerate(act_pairs):
                            prod = prpool.tile([128, BLK], F32R, bufs=1,
                                               tag=f"prod{pi % 3}", name="prod")
                            nc.vector.tensor_mul(
                                prod[:], qk_t[("q", m)][:], qk_t[("k", r)][:])
                            off = 62 - (16 * m + 2 * c)
                            i = c_sc[w]
                            c_sc[w] += 1
                            nc.tensor.matmul(
                                sc_ps[w][:], selw[:, off:off + 64], prod[:],
                                start=(i == 0), stop=(i == n_sc[w] - 1),
                                skip_group_check=True)

                    prev = emit_qk(0)
                    for c in range(1, NCH):
                        cur = emit_qk(c)
                        emit_scores(c - 1, prev)
                        prev = cur
                    emit_scores(NCH - 1, prev)
                    if pending_D:
                        emit_stage_D(*pending_D.pop(0))

                    # ---------- softmax ----------
                    s_sb = []
                    for w in range(WIN):
                        t = smpool.tile([64, BLK], F32, tag=f"s{w}")
                        if sc_ps[w] is None:
                            nc.gpsimd.memset(t[:], 0.0)
                        else:
                            nc.vector.tensor_copy(t[:], sc_ps[w][:])
                        s_sb.append(t)
                    mx = smpool.tile([64, BLK], F32, tag="mx")
                    nc.vector.tensor_tensor(mx[:], s_sb[0][:], s_sb[1][:], op=ALU.max)
                    nc.vector.tensor_tensor(mx[:], mx[:], s_sb[2][:], op=ALU.max)
                    attn = []
                    for w in range(WIN):
                        nc.vector.tensor_tensor(s_sb[w][:], s_sb[w][:], mx[:],
                                                op=ALU.subtract)
                        a = smpool.tile([64, BLK], F32, tag=f"at{w}")
                        nc.scalar.activation(a[:], s_sb[w][:], AF.Exp, scale=scale)
                        attn.append(a)
                    den = smpool.tile([64, BLK], F32, tag="mx")
                    nc.vector.tensor_add(den[:], attn[0][:], attn[1][:])
                    nc.vector.tensor_add(den[:], den[:], attn[2][:])
                    rec = smpool.tile([64, BLK], F32, tag="rec")
                    with nc.allow_low_precision(reason="attn weights"):
                        nc.vector.reciprocal(rec[:], den[:])
                    atb = []
                    for w in range(WIN):
                        ab = smpool.tile([64, BLK], BF16, tag=f"atb{w}")
                        nc.vector.tensor_mul(ab[:], attn[w][:], rec[:])
                        atb.append(ab)

                    # ---------- pass 2: v, A16, Abc, fused ----------
                    fz = fzpool.tile([128, NCH, BLK], BF16, tag="fz")

                    def emit_v(c):
                        v_t = {}
                        for (m, na_mm, nab) in cfg:
                            wv = wvpool.tile([128, NK[m], 128], MDT[m],
                                             bufs=(2 if m in (0, 1, 2) else 1),
                                             tag=f"wv{m}", name=f"wv{m}")
                            dma_engs[(c + m + 1) % 3].dma_start(wv[:], DV[m][c])
                            acc = nxt_acc()
                            for dk in range(NK[m]):
                                nc.tensor.matmul(
                                    acc[:, :na_mm], wv[:, dk, :],
                                    xt[(m, dk)][:, :na_mm],
                                    start=(dk == 0), stop=(dk == NK[m] - 1),
                                    skip_group_check=True)
                            t = qkpool.tile([128, BLK], F32,
                                            tag=f"v{m}_{c % 2}", name="vt")
                            nc.scalar.activation(
                                t[:, :nab], acc[:, :nab], AF.Identity,
                                bias=cqkv["cv"][:, m, c:c + 1])
                            if nab < BLK:
                                nc.vector.memset(t[:, nab:BLK], 0.0)
                            v_t[m] = t
                        return v_t

                    def emit_fused(c, v_t, a16sb):
                        ab_ps = {}
                        for ri, r in enumerate(act_r):
                            ab = (psum(5 + ri) if ri < 3 else nxt_acc())
                            nc.tensor.matmul(
                                ab[:], selB[:, c, :], a16sb[:, r, :],
                                start=True, stop=True,
                                skip_group_check=True)
                            ab_ps[r] = ab
                        ft = []
                        for j, r in enumerate(act_r):
                            t = prpool.tile([128, BLK], F32, tag=f"f{j}",
                                            name=f"f{j}")
                            nc.vector.tensor_mul(t[:], ab_ps[r][:], v_t[r][:])
                            ft.append(t)
                        if len(ft) == 1:
                            nc.vector.tensor_copy(fz[:, c, :], ft[0][:])
                        elif len(ft) == 2:
                            nc.vector.tensor_add(fz[:, c, :], ft[0][:], ft[1][:])
                        elif len(ft) == 3:
                            nc.vector.tensor_add(ft[0][:], ft[0][:], ft[1][:])
                            nc.vector.tensor_add(fz[:, c, :], ft[0][:], ft[2][:])
                        else:
                            nc.vector.tensor_add(ft[0][:], ft[0][:], ft[1][:])
                            nc.vector.tensor_add(ft[2][:], ft[2][:], ft[3][:])
                            nc.vector.tensor_add(fz[:, c, :], ft[0][:], ft[2][:])

                    vbuf = {cc: emit_v(cc) for cc in range(3)}

                    # A16 = per-source summed attn
                    a16sb = smpool.tile([16, M, BLK], BF16, tag="a16sb")
                    for ri, r in enumerate(act_r):
                        a16 = psum(5 + (ri % 2), (16, BLK))
                        srcs = SRC[r]
                        for i, (m, w) in enumerate(srcs):
                            nc.tensor.matmul(
                                a16[:], selA[:, PAIR_IDX[(m, w)], :],
                                atb[w][:],
                                start=(i == 0), stop=(i == len(srcs) - 1),
                                skip_group_check=True)
                        nc.scalar.activation(a16sb[:, r, :], a16[:], AF.Identity)

                    for c in range(NCH):
                        emit_fused(c, vbuf.pop(c), a16sb)
                        if c + 3 < NCH:
                            vbuf[c + 3] = emit_v(c + 3)

                    # ---------- stage D (deferred one block) ----------
                    def emit_stage_D(fz, p0):
                        for dc in range(NCH):
                            wsl = wopool.tile([128, NCH, 128], BF16,
                                              tag="wo", name="wsld")
                            dma_engs[dc % 3].dma_start(wsl[:], WoT[dc])
                            acc = nxt_acc()
                            for dk in range(NCH):
                                nc.tensor.matmul(
                                    acc[:], wsl[:, dk, :], fz[:, dk, :],
                                    start=(dk == 0), stop=(dk == NCH - 1),
                                    skip_group_check=True)
                            yo = yopool.tile([128, BLK], F32, tag="yo")
                            nc.scalar.activation(yo[:], acc[:], AF.Identity,
                                                 bias=bo_t[:, dc:dc + 1])
                            nc.sync.dma_start(
                                yT[dc * 128:(dc + 1) * 128, p0:p0 + BLK], yo[:])

                    pending_D.append((fz, p0))

                    if blk == NBLK - 1:
                        while pending_D:
                            emit_stage_D(*pending_D.pop(0))

    nc.compile()
    _BUILD_CACHE[key] = nc
    return nc


def make_selw():
    sw = np.zeros((128, 127), np.float32)
    for p in range(128):
        sw[p, 62 + p // 64] = 1.0
    return sw


def make_selA():
    sa = np.zeros((64, M * WIN, 16), np.float32)
    for m in range(M):
        for w in range(WIN):
            for h in range(16):
                sa[16 * m + h, m * WIN + w, h] = 1.0
    return sa


def make_selB():
    sb = np.zeros((16, NCH, 128), np.float32)
    for c in range(NCH):
        for j in range(128):
            sb[2 * c + j // 64, c, j] = 0.25
    return sb


def _vec_tile(v):
    return np.ascontiguousarray(np.asarray(v, np.float32).reshape(NCH, 128).T)


def _fold_w(WX, Wm):
    """(WX @ Wm).T -> [c, p, nk, j] layout for per-chunk stationary loads."""
    wt = (np.asarray(WX, np.float32) @ np.asarray(Wm, np.float32)).T  # [dim, D]
    nk = wt.shape[0] // 128
    wt = wt.reshape(nk, 128, NCH, 128)
    return np.ascontiguousarray(wt.transpose(2, 1, 0, 3))  # [c, p, nk, j]


def prepare_in_maps(inputs):
    names = [mm[0] for mm in MOD]
    shared = {}
    cq, ck, cv = [], [], []
    emb = np.asarray(inputs["mod_emb"], np.float32)
    for i, nm in enumerate(names):
        Wm = np.asarray(inputs[f"W_{nm}"], np.float32)   # [D, dim]
        qarr = _fold_w(inputs["Wq"], Wm)
        karr = _fold_w(inputs["Wk"], Wm)
        dqk = np.ascontiguousarray(np.stack([qarr, karr], axis=2))
        dv = _fold_w(inputs["Wv"], Wm)
        if i in (1, 3):
            dqk, dv = dqk.astype(BF16_NP), dv.astype(BF16_NP)
        shared[f"DQK{i}"] = dqk
        shared[f"DV{i}"] = dv
        ev = np.asarray(inputs[f"b_{nm}"], np.float32) + emb[i]
        cq.append(_vec_tile(np.asarray(inputs["Wq"], np.float32) @ ev
                            + np.asarray(inputs["bq"], np.float32)))
        ck.append(_vec_tile(np.asarray(inputs["Wk"], np.float32) @ ev
                            + np.asarray(inputs["bk"], np.float32)))
        cv.append(_vec_tile(np.asarray(inputs["Wv"], np.float32) @ ev
                            + np.asarray(inputs["bv"], np.float32)))
    shared["cq"] = np.ascontiguousarray(np.stack(cq, axis=1))
    shared["ck"] = np.ascontiguousarray(np.stack(ck, axis=1))
    shared["cv"] = np.ascontiguousarray(np.stack(cv, axis=1))

    wt = np.asarray(inputs["Wo"], np.float32).T          # [din, dout]
    wt = wt.reshape(NCH, 128, NCH, 128)
    shared["WoT"] = np.ascontiguousarray(wt.transpose(2, 1, 0, 3)).astype(BF16_NP)
    shared["bo"] = _vec_tile(inputs["bo"])
    shared["selw"] = make_selw()
    shared["selA"] = make_selA().astype(BF16_NP)
    shared["selB"] = make_selB().astype(BF16_NP)

    in_maps = []
    for core in range(8):
        b, par = core // 2, core % 2
        im = dict(shared)
        for i, nm in enumerate(names):
            x = np.asarray(inputs[nm], np.float32)[b, par::2][:NLOC[i]]
            xt = np.zeros((MOD[i][1], NLOCP[i]), np.float32)
            xt[:, :NLOC[i]] = x.T
            im[f"xT{i}"] = xt.astype(BF16_NP) if i in (1, 3) else xt
        in_maps.append(im)
    return in_maps


def kernel(**inputs):
    inputs = {k: np.asarray(v) for k, v in inputs.items()}
    scale = float(1.0 / (np.sqrt(HD) * abs(float(inputs["temperature"]))))
    nc = build(scale, repeat=1)
    in_maps = prepare_in_maps(inputs)
    res = run_bass_kernel_spmd(nc, in_maps, list(range(8)))
    out = np.zeros((B, S, D), np.float32)
    for core in range(8):
        b, par = core // 2, core % 2
        out[b, par::2, :] = res.results[core]["yT"].T
    return out
